# revision 4
# baseline (speedup 1.0000x reference)
"""Trainium2 Bass kernel: dot-product attention scoring + softmax.

Computes, for hidden [1, B, H] and encoder_outputs [S, B, H] (f32):
    energies[b, s] = <hidden[0, b, :], encoder_outputs[s, b, :]>
    out[b, 0, s]   = softmax(energies[b, :])   (softmax over s)

with B=32, S=4096, H=1024, sharded data-parallel over 8 NeuronCores
(4 batches per core; softmax is per-row so no collectives).

Strategy (memory-bound problem; the job is streaming 512 MiB of
encoder_outputs at full HBM bandwidth):
  - Host splits f32 E into an exact bf16 hi+lo pair (same total bytes as
    f32, ~1e-4 precision) so the TensorEngine can run at bf16 rate
    instead of the 4x-slower fp32 mode.
  - Per core, per batch b: energies chunk [1, 512] accumulates in PSUM
    over 3 matmul terms (h_hi*E_hi, h_hi*E_lo, h_lo*E_hi), stationary
    [128, 1] hidden columns, moving [128, 512] E tiles.
  - DVE copies each PSUM chunk to SBUF, fused with a running chunk-max;
    ACT does exp(x - max) with a fused sum; DVE scales by 1/sum.
"""

import os
import sys

import numpy as np

for _p in ("/opt/trn_rl_repo", "/root/.axon_site/_ro/trn_rl_repo"):
    if os.path.isdir(_p) and _p not in sys.path:
        sys.path.append(_p)

import ml_dtypes
from contextlib import ExitStack

import concourse.bass as bass
import concourse.tile as tile
from concourse import bacc, mybir
from concourse.bass_utils import run_bass_kernel_spmd

BF16 = ml_dtypes.bfloat16

# Problem constants (hardcoded per spec: nn_Attention_37529424232685)
S = 4096
B = 32
H = 1024
N_CORES = 8
B_L = B // N_CORES  # 4 batches per core


def build_nc(b_l=B_L, hc_n=H // 128, s=S, n_hf=2, sc=512, enable_asserts=False):
    """Build the per-core Bass program (SPMD: identical on all cores).

    DRAM inputs (per core):
      e_hi, e_lo : bf16 [b_l, hc_n, 128, s]   hi/lo split of E[b, h, s]
      hid        : bf16 [128, b_l * 2 * hc_n] column (b*2+plane)*hc_n+hc
                   holds hidden[b, hc*128+p] (plane 0 = hi, 1 = lo)
    DRAM output:
      out        : f32 [b_l, s] softmax weights
    """
    f32 = mybir.dt.float32
    bf16 = mybir.dt.bfloat16
    hc_per_hf = hc_n // n_hf
    nsc = s // sc

    nc = bacc.Bacc(
        "TRN2",
        target_bir_lowering=False,
        debug=False,
        enable_asserts=enable_asserts,
        num_devices=None,
    )

    e_hi = nc.dram_tensor("e_hi", [b_l, hc_n, 128, s], bf16, kind="ExternalInput").ap()
    e_lo = nc.dram_tensor("e_lo", [b_l, hc_n, 128, s], bf16, kind="ExternalInput").ap()
    hid = nc.dram_tensor("hid", [128, b_l * 2 * hc_n], bf16, kind="ExternalInput").ap()
    out = nc.dram_tensor("out", [b_l, s], f32, kind="ExternalOutput").ap()

    def col(b, plane, hc):
        return (b * 2 + plane) * hc_n + hc

    with tile.TileContext(nc) as tc, ExitStack() as ctx:
        mv_pool = ctx.enter_context(tc.tile_pool(name="mv", bufs=4))
        ps_pool = ctx.enter_context(
            tc.tile_pool(name="ps", bufs=min(8, nsc), space="PSUM")
        )
        en_pool = ctx.enter_context(tc.tile_pool(name="en", bufs=2))
        st_pool = ctx.enter_context(tc.tile_pool(name="st", bufs=2))
        c_pool = ctx.enter_context(tc.tile_pool(name="const", bufs=1))

        hid_t = c_pool.tile([128, b_l * 2 * hc_n], bf16, name="hid_t")
        nc.sync.dma_start(out=hid_t[:], in_=hid[:])

        # phases: (plane, half) — plane 0 tiles (E_hi) are hit by two
        # stationaries (h_hi, h_lo); plane 1 tiles (E_lo) by h_hi only.
        phases = [(0, hf) for hf in range(n_hf)] + [(1, hf) for hf in range(n_hf)]

        for b in range(b_l):
            mv_tiles = {}
            for pl, hf in phases:
                mv = mv_pool.tile([128, hc_per_hf, s], bf16, name="mv", tag="mv")
                src = e_hi if pl == 0 else e_lo
                nc.sync.dma_start(
                    out=mv[:],
                    in_=src[b][hf * hc_per_hf : (hf + 1) * hc_per_hf].rearrange(
                        "h p s -> p h s"
                    ),
                )
                mv_tiles[(pl, hf)] = mv

            ps_tiles = [
                ps_pool.tile([1, sc], f32, name="ps", tag="ps") for _ in range(nsc)
            ]

            for pl, hf in phases:
                mv = mv_tiles[(pl, hf)]
                stat_planes = (0, 1) if pl == 0 else (0,)
                for isc in range(nsc):
                    for hcl in range(hc_per_hf):
                        hc = hf * hc_per_hf + hcl
                        for stp in stat_planes:
                            first = pl == 0 and hf == 0 and hcl == 0 and stp == 0
                            last = pl == 1 and hf == n_hf - 1 and hcl == hc_per_hf - 1
                            c = col(b, stp, hc)
                            nc.tensor.matmul(
                                ps_tiles[isc][:],
                                lhsT=hid_t[:, c : c + 1],
                                rhs=mv[:, hcl, bass.ts(isc, sc)],
                                start=first,
                                stop=last,
                            )

            energ = en_pool.tile([1, s], f32, name="energ", tag="energ")
            maxes = st_pool.tile([1, nsc], f32, name="maxes", tag="maxes")
            for isc in range(nsc):
                # PSUM chunk -> SBUF (ACT), then chunk max (DVE, from SBUF
                # so the PSUM bank frees after the copy alone)
                nc.scalar.copy(energ[0:1, bass.ts(isc, sc)], ps_tiles[isc][:])
                nc.vector.tensor_reduce(
                    out=maxes[0:1, isc : isc + 1],
                    in_=energ[0:1, bass.ts(isc, sc)],
                    axis=mybir.AxisListType.X,
                    op=mybir.AluOpType.max,
                )

            negmax = st_pool.tile([1, 1], f32, name="negmax", tag="negmax")
            nc.vector.tensor_reduce(
                out=negmax[:],
                in_=maxes[:],
                axis=mybir.AxisListType.X,
                op=mybir.AluOpType.max,
                negate=True,
            )
            sumexp = st_pool.tile([1, 1], f32, name="sumexp", tag="sumexp")
            nc.scalar.activation(
                out=energ[:],
                in_=energ[:],
                func=mybir.ActivationFunctionType.Exp,
                bias=negmax[0:1, 0:1],
                scale=1.0,
                accum_out=sumexp[0:1, 0:1],
            )
            inv = st_pool.tile([1, 1], f32, name="inv", tag="inv")
            nc.vector.reciprocal(inv[:], sumexp[:])
            nc.vector.tensor_scalar_mul(energ[:], energ[:], inv[0:1, 0:1])
            nc.scalar.dma_start(out=out[b : b + 1, :], in_=energ[:])

    nc.compile()
    return nc


def split_hi_lo(x_f32):
    """Exact-ish decomposition x ~= hi + lo with hi, lo bf16."""
    hi = x_f32.astype(BF16)
    lo = (x_f32 - hi.astype(np.float32)).astype(BF16)
    return hi, lo


def make_in_maps(hidden, encoder_outputs):
    """Shard + lay out host-side. hidden [1,B,H] f32, enc [S,B,H] f32."""
    hc_n = H // 128
    in_maps = []
    for i in range(N_CORES):
        b0 = i * B_L
        # E per core: [b_l, H, S] (b, h, s)
        E = np.ascontiguousarray(
            encoder_outputs[:, b0 : b0 + B_L, :].transpose(1, 2, 0)
        ).astype(np.float32, copy=False)
        e_hi, e_lo = split_hi_lo(E)
        e_hi = e_hi.reshape(B_L, hc_n, 128, S)
        e_lo = e_lo.reshape(B_L, hc_n, 128, S)

        hs = hidden[0, b0 : b0 + B_L, :].astype(np.float32, copy=False)  # [b_l, H]
        h_hi, h_lo = split_hi_lo(hs)
        # hid[p, (b*2+plane)*hc_n + hc] = h_plane[b, hc*128+p]
        hh = h_hi.reshape(B_L, hc_n, 128)
        hl = h_lo.reshape(B_L, hc_n, 128)
        stacked = np.stack([hh, hl], axis=1)  # [b_l, 2, hc_n, 128]
        hid_arr = np.ascontiguousarray(
            stacked.transpose(3, 0, 1, 2).reshape(128, B_L * 2 * hc_n)
        )
        in_maps.append({"e_hi": e_hi, "e_lo": e_lo, "hid": hid_arr})
    return in_maps


_NC_CACHE = {}


def _get_nc():
    if "nc" not in _NC_CACHE:
        _NC_CACHE["nc"] = build_nc()
    return _NC_CACHE["nc"]


def run(hidden, encoder_outputs, trace=False, trace_cores=None):
    """Returns (output [B, 1, S] f32, BassKernelResults)."""
    hidden = np.asarray(hidden)
    encoder_outputs = np.asarray(encoder_outputs)
    nc = _get_nc()
    in_maps = make_in_maps(hidden, encoder_outputs)
    res = run_bass_kernel_spmd(
        nc,
        in_maps,
        core_ids=list(range(N_CORES)),
        trace=trace,
        trace_cores=trace_cores,
    )
    full = np.empty((B, S), dtype=np.float32)
    for i in range(N_CORES):
        full[i * B_L : (i + 1) * B_L] = res.results[i]["out"]
    return full.reshape(B, 1, S), res


def kernel(hidden, encoder_outputs):
    out, _ = run(hidden, encoder_outputs, trace=False)
    return out


# revision 15
# speedup vs baseline: 1.0042x; 1.0042x over previous
"""Trainium2 Bass kernel: dot-product attention scoring + softmax.

Computes, for hidden [1, B, H] and encoder_outputs [S, B, H] (f32):
    energies[b, s] = <hidden[0, b, :], encoder_outputs[s, b, :]>
    out[b, 0, s]   = softmax(energies[b, :])   (softmax over s)

with B=32, S=4096, H=1024, sharded data-parallel over 8 NeuronCores
(4 batches per core; softmax is per-row so no collectives).

Strategy (memory-bound problem; the job is streaming 512 MiB of
encoder_outputs at full HBM bandwidth):
  - Host splits f32 E into an exact bf16 hi+lo pair (same total bytes as
    f32, ~1e-4 precision) so the TensorEngine can run at bf16 rate
    instead of the 4x-slower fp32 mode.
  - Per core, per batch b: energies chunk [1, 512] accumulates in PSUM
    over 3 matmul terms (h_hi*E_hi, h_hi*E_lo, h_lo*E_hi), stationary
    [128, 1] hidden columns, moving [128, 512] E tiles.
  - DVE copies each PSUM chunk to SBUF, fused with a running chunk-max;
    ACT does exp(x - max) with a fused sum; DVE scales by 1/sum.
"""

import os
import sys

import numpy as np

for _p in ("/opt/trn_rl_repo", "/root/.axon_site/_ro/trn_rl_repo"):
    if os.path.isdir(_p) and _p not in sys.path:
        sys.path.append(_p)

import ml_dtypes
from contextlib import ExitStack

import concourse.bass as bass
import concourse.tile as tile
from concourse import bacc, mybir
from concourse.bass_utils import run_bass_kernel_spmd

BF16 = ml_dtypes.bfloat16

# Problem constants (hardcoded per spec: nn_Attention_37529424232685)
S = 4096
B = 32
H = 1024
N_CORES = 8
B_L = B // N_CORES  # 4 batches per core


def build_nc(b_l=B_L, hc_n=H // 128, s=S, n_hf=2, sc=512, enable_asserts=False):
    """Build the per-core Bass program (SPMD: identical on all cores).

    DRAM inputs (per core):
      e_hi, e_lo : bf16 [b_l, hc_n, 128, s]   hi/lo split of E[b, h, s]
      hid        : bf16 [128, b_l * 2 * hc_n] column (b*2+plane)*hc_n+hc
                   holds hidden[b, hc*128+p] (plane 0 = hi, 1 = lo)
    DRAM output:
      out        : f32 [b_l, s] softmax weights
    """
    f32 = mybir.dt.float32
    bf16 = mybir.dt.bfloat16
    hc_per_hf = hc_n // n_hf
    nsc = s // sc

    nc = bacc.Bacc(
        "TRN2",
        target_bir_lowering=False,
        debug=False,
        enable_asserts=enable_asserts,
        num_devices=None,
    )

    e_hi = nc.dram_tensor("e_hi", [b_l, hc_n, 128, s], bf16, kind="ExternalInput").ap()
    e_lo = nc.dram_tensor("e_lo", [b_l, hc_n, 128, s], bf16, kind="ExternalInput").ap()
    hid = nc.dram_tensor("hid", [128, b_l * 2 * hc_n], bf16, kind="ExternalInput").ap()
    sel01 = nc.dram_tensor("sel01", [2, 1], bf16, kind="ExternalInput").ap()
    out = nc.dram_tensor("out", [b_l, s], f32, kind="ExternalOutput").ap()

    def col(b, hc):
        # column pair (h_hi, h_lo) for stationary [128, 2]
        return (b * hc_n + hc) * 2

    with tile.TileContext(nc) as tc, ExitStack() as ctx:
        mv_pool = ctx.enter_context(tc.tile_pool(name="mv", bufs=4))
        ps_pool = ctx.enter_context(
            tc.tile_pool(name="ps", bufs=min(8, nsc), space="PSUM")
        )
        en_pool = ctx.enter_context(tc.tile_pool(name="en", bufs=2))
        st_pool = ctx.enter_context(tc.tile_pool(name="st", bufs=2))
        z2_pool = ctx.enter_context(tc.tile_pool(name="z2", bufs=4))
        c_pool = ctx.enter_context(tc.tile_pool(name="const", bufs=1))

        hid_t = c_pool.tile([128, b_l * 2 * hc_n], bf16, name="hid_t")
        nc.sync.dma_start(out=hid_t[:], in_=hid[:])
        # collapse stationary [[0],[1]]: psum row0 += 0*z2row0 + 1*z2row1
        ones_t = c_pool.tile([2, 1], bf16, name="ones_t")
        nc.sync.dma_start(out=ones_t[:], in_=sel01[:])

        # phases: (plane, half) — plane 0 tiles (E_hi) are hit by two
        # stationaries (h_hi, h_lo); plane 1 tiles (E_lo) by h_hi only.
        phases = [(0, hf) for hf in range(n_hf)] + [(1, hf) for hf in range(n_hf)]

        for b in range(b_l):
            mv_tiles = {}
            for pl, hf in phases:
                mv = mv_pool.tile([128, hc_per_hf, s], bf16, name="mv", tag="mv")
                src = e_hi if pl == 0 else e_lo
                nc.sync.dma_start(
                    out=mv[:],
                    in_=src[b][hf * hc_per_hf : (hf + 1) * hc_per_hf].rearrange(
                        "h p s -> p h s"
                    ),
                )
                mv_tiles[(pl, hf)] = mv

            ps_tiles = [
                ps_pool.tile([2, sc], f32, name="ps", tag="ps") for _ in range(nsc)
            ]

            for pl, hf in phases:
                mv = mv_tiles[(pl, hf)]
                for isc in range(nsc):
                    for hcl in range(hc_per_hf):
                        hc = hf * hc_per_hf + hcl
                        first = pl == 0 and hf == 0 and hcl == 0
                        last = pl == 1 and hf == n_hf - 1 and hcl == hc_per_hf - 1
                        c = col(b, hc)
                        # row0 += h_hi * E_pl ; row1 += h_lo * E_pl
                        nc.tensor.matmul(
                            ps_tiles[isc][:],
                            lhsT=hid_t[:, c : c + 2],
                            rhs=mv[:, hcl, bass.ts(isc, sc)],
                            start=first,
                            stop=last,
                        )

            energ = en_pool.tile([1, s], f32, name="energ", tag="energ")
            maxes = st_pool.tile([1, nsc], f32, name="maxes", tag="maxes")
            for isc in range(nsc):
                # Fold correction row1 into row0: copy psum pair to bf16
                # (row1 is a small correction so bf16 suffices; row0's copy
                # is multiplied by 0), then row0 += [0,1]^T @ z2.
                z2 = z2_pool.tile([2, sc], bf16, name="z2", tag="z2")
                nc.scalar.copy(z2[:], ps_tiles[isc][:])
                # start=False: has_written persists from the closed group, so
                # this accumulates row1's correction onto row0 in place.
                nc.tensor.matmul(
                    ps_tiles[isc][0:1, :],
                    lhsT=ones_t[:],
                    rhs=z2[:],
                    start=False,
                    stop=True,
                    skip_group_check=True,
                )
                # PSUM row0 -> SBUF (ACT), then chunk max (DVE, from SBUF
                # so the PSUM bank frees after the copy alone)
                nc.scalar.copy(energ[0:1, bass.ts(isc, sc)], ps_tiles[isc][0:1, :])
                nc.vector.tensor_reduce(
                    out=maxes[0:1, isc : isc + 1],
                    in_=energ[0:1, bass.ts(isc, sc)],
                    axis=mybir.AxisListType.X,
                    op=mybir.AluOpType.max,
                )

            negmax = st_pool.tile([1, 1], f32, name="negmax", tag="negmax")
            nc.vector.tensor_reduce(
                out=negmax[:],
                in_=maxes[:],
                axis=mybir.AxisListType.X,
                op=mybir.AluOpType.max,
                negate=True,
            )
            sumexp = st_pool.tile([1, 1], f32, name="sumexp", tag="sumexp")
            nc.scalar.activation(
                out=energ[:],
                in_=energ[:],
                func=mybir.ActivationFunctionType.Exp,
                bias=negmax[0:1, 0:1],
                scale=1.0,
                accum_out=sumexp[0:1, 0:1],
            )
            inv = st_pool.tile([1, 1], f32, name="inv", tag="inv")
            nc.vector.reciprocal(inv[:], sumexp[:])
            nc.vector.tensor_scalar_mul(energ[:], energ[:], inv[0:1, 0:1])
            nc.scalar.dma_start(out=out[b : b + 1, :], in_=energ[:])

    nc.compile()
    return nc


def split_hi_lo(x_f32):
    """Exact-ish decomposition x ~= hi + lo with hi, lo bf16."""
    hi = x_f32.astype(BF16)
    lo = (x_f32 - hi.astype(np.float32)).astype(BF16)
    return hi, lo


def make_core_inputs(E, hs, hc_n, s):
    """Per-core input map from E [b_l, H, s] f32 and hs [b_l, H] f32."""
    b_l = E.shape[0]
    e_hi, e_lo = split_hi_lo(E)
    h_hi, h_lo = split_hi_lo(hs)
    hh = h_hi.reshape(b_l, hc_n, 128)
    hl = h_lo.reshape(b_l, hc_n, 128)
    stacked = np.stack([hh, hl], axis=-1)  # [b_l, hc_n, 128, 2]
    hid_arr = np.ascontiguousarray(
        stacked.transpose(2, 0, 1, 3).reshape(128, b_l * 2 * hc_n)
    )
    return {
        "e_hi": e_hi.reshape(b_l, hc_n, 128, s),
        "e_lo": e_lo.reshape(b_l, hc_n, 128, s),
        "hid": hid_arr,
        "sel01": np.array([[0.0], [1.0]], dtype=BF16),
    }


def make_in_maps(hidden, encoder_outputs):
    """Shard + lay out host-side. hidden [1,B,H] f32, enc [S,B,H] f32."""
    hc_n = H // 128
    in_maps = []
    for i in range(N_CORES):
        b0 = i * B_L
        # E per core: [b_l, H, S] (b, h, s)
        E = np.ascontiguousarray(
            encoder_outputs[:, b0 : b0 + B_L, :].transpose(1, 2, 0)
        ).astype(np.float32, copy=False)
        hs = hidden[0, b0 : b0 + B_L, :].astype(np.float32, copy=False)
        in_maps.append(make_core_inputs(E, hs, hc_n, S))
    return in_maps


_NC_CACHE = {}


def _get_nc():
    if "nc" not in _NC_CACHE:
        _NC_CACHE["nc"] = build_nc()
    return _NC_CACHE["nc"]


def run(hidden, encoder_outputs, trace=False, trace_cores=None):
    """Returns (output [B, 1, S] f32, BassKernelResults)."""
    hidden = np.asarray(hidden)
    encoder_outputs = np.asarray(encoder_outputs)
    nc = _get_nc()
    in_maps = make_in_maps(hidden, encoder_outputs)
    res = run_bass_kernel_spmd(
        nc,
        in_maps,
        core_ids=list(range(N_CORES)),
        trace=trace,
        trace_cores=trace_cores,
    )
    full = np.empty((B, S), dtype=np.float32)
    for i in range(N_CORES):
        full[i * B_L : (i + 1) * B_L] = res.results[i]["out"]
    return full.reshape(B, 1, S), res


def kernel(hidden, encoder_outputs):
    out, _ = run(hidden, encoder_outputs, trace=False)
    return out


# revision 17
# speedup vs baseline: 1.1877x; 1.1826x over previous
"""Trainium2 Bass kernel: dot-product attention scoring + softmax.

Computes, for hidden [1, B, H] and encoder_outputs [S, B, H] (f32):
    energies[b, s] = <hidden[0, b, :], encoder_outputs[s, b, :]>
    out[b, 0, s]   = softmax(energies[b, :])   (softmax over s)

with B=32, S=4096, H=1024, sharded data-parallel over 8 NeuronCores
(4 batches per core; softmax is per-row so no collectives).

Strategy (memory-bound problem; the job is streaming 512 MiB of
encoder_outputs at full HBM bandwidth):
  - Host splits f32 E into an exact bf16 hi+lo pair (same total bytes as
    f32, ~1e-4 precision) so the TensorEngine can run at bf16 rate
    instead of the 4x-slower fp32 mode.
  - Per core, per batch b: energies chunk [1, 512] accumulates in PSUM
    over 3 matmul terms (h_hi*E_hi, h_hi*E_lo, h_lo*E_hi), stationary
    [128, 1] hidden columns, moving [128, 512] E tiles.
  - DVE copies each PSUM chunk to SBUF, fused with a running chunk-max;
    ACT does exp(x - max) with a fused sum; DVE scales by 1/sum.
"""

import os
import sys

import numpy as np

for _p in ("/opt/trn_rl_repo", "/root/.axon_site/_ro/trn_rl_repo"):
    if os.path.isdir(_p) and _p not in sys.path:
        sys.path.append(_p)

import ml_dtypes
from contextlib import ExitStack

import concourse.bass as bass
import concourse.tile as tile
from concourse import bacc, mybir
from concourse.bass_utils import run_bass_kernel_spmd

BF16 = ml_dtypes.bfloat16

# Problem constants (hardcoded per spec: nn_Attention_37529424232685)
S = 4096
B = 32
H = 1024
N_CORES = 8
B_L = B // N_CORES  # 4 batches per core


def build_nc(b_l=B_L, hc_n=H // 128, s=S, n_hf=4, sc=512, enable_asserts=False):
    """Build the per-core Bass program (SPMD: identical on all cores).

    DRAM inputs (per core):
      e_hi, e_lo : bf16 [b_l, hc_n, 128, s]   hi/lo split of E[b, h, s]
      hid        : bf16 [128, b_l * 2 * hc_n] column (b*2+plane)*hc_n+hc
                   holds hidden[b, hc*128+p] (plane 0 = hi, 1 = lo)
    DRAM output:
      out        : f32 [b_l, s] softmax weights
    """
    f32 = mybir.dt.float32
    bf16 = mybir.dt.bfloat16
    hc_per_hf = hc_n // n_hf
    nsc = s // sc

    nc = bacc.Bacc(
        "TRN2",
        target_bir_lowering=False,
        debug=False,
        enable_asserts=enable_asserts,
        num_devices=None,
    )

    e_hi = nc.dram_tensor("e_hi", [b_l, hc_n, 128, s], bf16, kind="ExternalInput").ap()
    e_lo = nc.dram_tensor("e_lo", [b_l, hc_n, 128, s], bf16, kind="ExternalInput").ap()
    hid = nc.dram_tensor("hid", [128, b_l * 2 * hc_n], bf16, kind="ExternalInput").ap()
    sel01 = nc.dram_tensor("sel01", [2, 1], bf16, kind="ExternalInput").ap()
    out = nc.dram_tensor("out", [b_l, s], f32, kind="ExternalOutput").ap()

    def col(b, hc):
        # column pair (h_hi, h_lo) for stationary [128, 2]
        return (b * hc_n + hc) * 2

    with tile.TileContext(nc) as tc, ExitStack() as ctx:
        mv_pool = ctx.enter_context(tc.tile_pool(name="mv", bufs=2 * n_hf))
        ps_pool = ctx.enter_context(
            tc.tile_pool(name="ps", bufs=min(8, nsc), space="PSUM")
        )
        en_pool = ctx.enter_context(tc.tile_pool(name="en", bufs=2))
        st_pool = ctx.enter_context(tc.tile_pool(name="st", bufs=2))
        z2_pool = ctx.enter_context(tc.tile_pool(name="z2", bufs=4))
        c_pool = ctx.enter_context(tc.tile_pool(name="const", bufs=1))

        hid_t = c_pool.tile([128, b_l * 2 * hc_n], bf16, name="hid_t")
        nc.sync.dma_start(out=hid_t[:], in_=hid[:])
        # collapse stationary [[0],[1]]: psum row0 += 0*z2row0 + 1*z2row1
        ones_t = c_pool.tile([2, 1], bf16, name="ones_t")
        nc.sync.dma_start(out=ones_t[:], in_=sel01[:])

        # phases: (plane, half) — plane 0 tiles (E_hi) are hit by two
        # stationaries (h_hi, h_lo); plane 1 tiles (E_lo) by h_hi only.
        phases = [(0, hf) for hf in range(n_hf)] + [(1, hf) for hf in range(n_hf)]

        for b in range(b_l):
            mv_tiles = {}
            for pl, hf in phases:
                mv = mv_pool.tile([128, hc_per_hf, s], bf16, name="mv", tag="mv")
                src = e_hi if pl == 0 else e_lo
                nc.sync.dma_start(
                    out=mv[:],
                    in_=src[b][hf * hc_per_hf : (hf + 1) * hc_per_hf].rearrange(
                        "h p s -> p h s"
                    ),
                )
                mv_tiles[(pl, hf)] = mv

            ps_tiles = [
                ps_pool.tile([2, sc], f32, name="ps", tag="ps") for _ in range(nsc)
            ]

            for pl, hf in phases:
                mv = mv_tiles[(pl, hf)]
                for isc in range(nsc):
                    for hcl in range(hc_per_hf):
                        hc = hf * hc_per_hf + hcl
                        first = pl == 0 and hf == 0 and hcl == 0
                        last = pl == 1 and hf == n_hf - 1 and hcl == hc_per_hf - 1
                        c = col(b, hc)
                        # row0 += h_hi * E_pl ; row1 += h_lo * E_pl
                        nc.tensor.matmul(
                            ps_tiles[isc][:],
                            lhsT=hid_t[:, c : c + 2],
                            rhs=mv[:, hcl, bass.ts(isc, sc)],
                            start=first,
                            stop=last,
                        )

            energ = en_pool.tile([1, s], f32, name="energ", tag="energ")
            maxes = st_pool.tile([1, nsc], f32, name="maxes", tag="maxes")
            for isc in range(nsc):
                # Fold correction row1 into row0: copy psum pair to bf16
                # (row1 is a small correction so bf16 suffices; row0's copy
                # is multiplied by 0), then row0 += [0,1]^T @ z2.
                z2 = z2_pool.tile([2, sc], bf16, name="z2", tag="z2")
                nc.scalar.copy(z2[:], ps_tiles[isc][:])
                # start=False: has_written persists from the closed group, so
                # this accumulates row1's correction onto row0 in place.
                nc.tensor.matmul(
                    ps_tiles[isc][0:1, :],
                    lhsT=ones_t[:],
                    rhs=z2[:],
                    start=False,
                    stop=True,
                    skip_group_check=True,
                )
                # PSUM row0 -> SBUF (ACT), then chunk max (DVE, from SBUF
                # so the PSUM bank frees after the copy alone)
                nc.scalar.copy(energ[0:1, bass.ts(isc, sc)], ps_tiles[isc][0:1, :])
                nc.vector.tensor_reduce(
                    out=maxes[0:1, isc : isc + 1],
                    in_=energ[0:1, bass.ts(isc, sc)],
                    axis=mybir.AxisListType.X,
                    op=mybir.AluOpType.max,
                )

            negmax = st_pool.tile([1, 1], f32, name="negmax", tag="negmax")
            nc.vector.tensor_reduce(
                out=negmax[:],
                in_=maxes[:],
                axis=mybir.AxisListType.X,
                op=mybir.AluOpType.max,
                negate=True,
            )
            sumexp = st_pool.tile([1, 1], f32, name="sumexp", tag="sumexp")
            nc.scalar.activation(
                out=energ[:],
                in_=energ[:],
                func=mybir.ActivationFunctionType.Exp,
                bias=negmax[0:1, 0:1],
                scale=1.0,
                accum_out=sumexp[0:1, 0:1],
            )
            inv = st_pool.tile([1, 1], f32, name="inv", tag="inv")
            nc.vector.reciprocal(inv[:], sumexp[:])
            nc.vector.tensor_scalar_mul(energ[:], energ[:], inv[0:1, 0:1])
            nc.scalar.dma_start(out=out[b : b + 1, :], in_=energ[:])

    nc.compile()
    return nc


def split_hi_lo(x_f32):
    """Exact-ish decomposition x ~= hi + lo with hi, lo bf16."""
    hi = x_f32.astype(BF16)
    lo = (x_f32 - hi.astype(np.float32)).astype(BF16)
    return hi, lo


def make_core_inputs(E, hs, hc_n, s):
    """Per-core input map from E [b_l, H, s] f32 and hs [b_l, H] f32."""
    b_l = E.shape[0]
    e_hi, e_lo = split_hi_lo(E)
    h_hi, h_lo = split_hi_lo(hs)
    hh = h_hi.reshape(b_l, hc_n, 128)
    hl = h_lo.reshape(b_l, hc_n, 128)
    stacked = np.stack([hh, hl], axis=-1)  # [b_l, hc_n, 128, 2]
    hid_arr = np.ascontiguousarray(
        stacked.transpose(2, 0, 1, 3).reshape(128, b_l * 2 * hc_n)
    )
    return {
        "e_hi": e_hi.reshape(b_l, hc_n, 128, s),
        "e_lo": e_lo.reshape(b_l, hc_n, 128, s),
        "hid": hid_arr,
        "sel01": np.array([[0.0], [1.0]], dtype=BF16),
    }


def make_in_maps(hidden, encoder_outputs):
    """Shard + lay out host-side. hidden [1,B,H] f32, enc [S,B,H] f32."""
    hc_n = H // 128
    in_maps = []
    for i in range(N_CORES):
        b0 = i * B_L
        # E per core: [b_l, H, S] (b, h, s)
        E = np.ascontiguousarray(
            encoder_outputs[:, b0 : b0 + B_L, :].transpose(1, 2, 0)
        ).astype(np.float32, copy=False)
        hs = hidden[0, b0 : b0 + B_L, :].astype(np.float32, copy=False)
        in_maps.append(make_core_inputs(E, hs, hc_n, S))
    return in_maps


_NC_CACHE = {}


def _get_nc():
    if "nc" not in _NC_CACHE:
        _NC_CACHE["nc"] = build_nc()
    return _NC_CACHE["nc"]


def run(hidden, encoder_outputs, trace=False, trace_cores=None):
    """Returns (output [B, 1, S] f32, BassKernelResults)."""
    hidden = np.asarray(hidden)
    encoder_outputs = np.asarray(encoder_outputs)
    nc = _get_nc()
    in_maps = make_in_maps(hidden, encoder_outputs)
    res = run_bass_kernel_spmd(
        nc,
        in_maps,
        core_ids=list(range(N_CORES)),
        trace=trace,
        trace_cores=trace_cores,
    )
    full = np.empty((B, S), dtype=np.float32)
    for i in range(N_CORES):
        full[i * B_L : (i + 1) * B_L] = res.results[i]["out"]
    return full.reshape(B, 1, S), res


def kernel(hidden, encoder_outputs):
    out, _ = run(hidden, encoder_outputs, trace=False)
    return out
